# revision 6
# baseline (speedup 1.0000x reference)
"""Multi-head self-attention kernel for 8 Trainium2 NeuronCores.

Sharding: core c = (b, g) with b = batch index (4), g = head-group (2).
Each core computes attention for one batch element and 8 of the 16 heads,
including its slice of the QKV projections and a partial out-projection
(Y_partial = O_heads @ Wo[rows of its heads]).  The host sums the two
head-group partials per batch and transposes (the device produces Y^T).

On-device layout is fully "transposed": x^T [D, S] in, Q^T/K^T [dk, S],
scores S^T = K_h Q_h^T [k, q] (softmax along partitions via a ones-column
appended to V: the PV matmul O^T_aug = [V|1]^T P^T yields the softmax
denominator in its last row), output Y^T [D, S].
"""

import sys

sys.path.insert(0, "/opt/trn_rl_repo")

from contextlib import ExitStack

import numpy as np

import concourse.bass as bass
import concourse.tile as tile
from concourse import bacc, mybir
from concourse.bass_utils import run_bass_kernel_spmd

F32 = mybir.dt.float32
F32R = mybir.dt.float32r
P = 128  # SBUF partitions

D_MODEL = 1024
NHEAD = 16
DK = D_MODEL // NHEAD  # 64
BATCH = 4
SEQ = 2048
N_CORES = 8
HL = NHEAD // 2  # heads per core (head-group of 8)


def build_bass(D=D_MODEL, S=SEQ, HLOC=HL, QB=512):
    """Build the per-core Bass program (same program on all 8 cores)."""
    DC = D // P           # d_model chunks (contraction for projections)
    KC = S // P           # key chunks
    NQB = S // QB         # q blocks
    NPAIR = HLOC // 2     # head pairs
    HD = HLOC * DK        # local head dim total (512)
    VW = DK + 1           # V columns per head incl. ones column
    NOC = D // P          # out-dim chunks
    EXP_SCALE = 1.0 / np.sqrt(DK)

    nc = bacc.Bacc("TRN2", target_bir_lowering=False, debug=False,
                   num_devices=N_CORES)

    xT = nc.dram_tensor("xT", [D, S], F32, kind="ExternalInput")
    Wq = nc.dram_tensor("Wq", [D, HD], F32, kind="ExternalInput")
    Wk = nc.dram_tensor("Wk", [D, HD], F32, kind="ExternalInput")
    Wv = nc.dram_tensor("Wv", [D, HD], F32, kind="ExternalInput")
    Wo = nc.dram_tensor("Wo", [HD, D], F32, kind="ExternalInput")
    bq_t = nc.dram_tensor("bq_t", [P, NPAIR], F32, kind="ExternalInput")
    bk_t = nc.dram_tensor("bk_t", [P, NPAIR], F32, kind="ExternalInput")
    bv_bc = nc.dram_tensor("bv_bc", [P, HD], F32, kind="ExternalInput")
    bo_t = nc.dram_tensor("bo_t", [P, NOC], F32, kind="ExternalInput")
    YT = nc.dram_tensor("YT", [D, S], F32, kind="ExternalOutput")

    with tile.TileContext(nc) as tc, ExitStack() as ctx:
        consts = ctx.enter_context(tc.tile_pool(name="consts", bufs=1))
        ktv = ctx.enter_context(tc.tile_pool(name="ktv", bufs=1))
        ps_a = ctx.enter_context(tc.tile_pool(name="ps_a", bufs=2, space="PSUM"))
        ps_b = ctx.enter_context(tc.tile_pool(name="ps_b", bufs=2, space="PSUM"))
        ps_o = ctx.enter_context(tc.tile_pool(name="ps_o", bufs=1, space="PSUM"))
        ps_o2 = ctx.enter_context(tc.tile_pool(name="ps_o2", bufs=1, space="PSUM"))
        ps_y = ctx.enter_context(tc.tile_pool(name="ps_y", bufs=2, space="PSUM"))

        # ---- constants ----
        bq_sb = consts.tile([P, NPAIR], F32, tag="bq")
        bk_sb = consts.tile([P, NPAIR], F32, tag="bk")
        bv_sb = consts.tile([P, HD], F32, tag="bv")
        bo_sb = consts.tile([P, NOC], F32, tag="bo")
        nc.sync.dma_start(bq_sb[:], bq_t.ap())
        nc.sync.dma_start(bk_sb[:], bk_t.ap())
        nc.sync.dma_start(bv_sb[:], bv_bc.ap())
        nc.sync.dma_start(bo_sb[:], bo_t.ap())

        # warm the ACT exp table early
        warm = consts.tile([1, 2], F32, tag="warm")
        nc.gpsimd.memset(warm[0:1, 0:1], 0.0)
        nc.scalar.activation(warm[0:1, 1:2], warm[0:1, 0:1],
                             mybir.ActivationFunctionType.Exp)

        ones_sb = consts.tile([P, HLOC], F32, tag="ones")
        nc.vector.memset(ones_sb[:], 1.0)

        # resident outputs of phase 1
        qt_tiles = [ktv.tile([P, S], F32R, tag=f"qt{p_}", name=f"qt{p_}")
                    for p_ in range(NPAIR)]
        kt_tiles = [ktv.tile([P, S], F32R, tag=f"kt{p_}", name=f"kt{p_}")
                    for p_ in range(NPAIR)]
        v_tiles = [ktv.tile([P, HLOC * VW], F32R, tag=f"v{k}", name=f"v{k}")
                   for k in range(KC)]

        xt_dram3 = xT.ap().bitcast(F32R).rearrange("(c p) s -> p c s", p=P)

        # ---- phase 1: Q^T, K^T, V projections (x^T + W streamed) ----
        with tc.tile_pool(name="wqkv", bufs=1) as wpool, \
             tc.tile_pool(name="xwin", bufs=2) as xpool:

            def load_w(name, dram, cols):
                t = wpool.tile([P, DC * cols], F32R, tag=name, name=name)
                nc.sync.dma_start(
                    t[:].rearrange("p (c n) -> p c n", c=DC),
                    dram.ap().bitcast(F32R).rearrange("(c p) n -> p c n", p=P))
                return t

            wq_sb = load_w("wq", Wq, HD)
            wk_sb = load_w("wk", Wk, HD)
            wv_sb = load_w("wv", Wv, HD)

            def wslice(wt, c, lo, hi):
                return wt[:, c * HD + lo: c * HD + hi]

            for w in range(NQB):
                sl = bass.ts(w, QB)
                xw = xpool.tile([P, DC * QB], F32R, tag="xw")
                xw3 = xw[:].rearrange("p (c s) -> p c s", c=DC)
                nc.sync.dma_start(xw3[:], xt_dram3[:, :, sl])

                for pr in range(NPAIR):
                    qps = ps_a.tile([P, QB], F32, tag="sa")
                    kps = ps_b.tile([P, QB], F32, tag="sb")
                    for c in range(DC):
                        nc.tensor.matmul(qps[:],
                                         wslice(wq_sb, c, pr * P, (pr + 1) * P),
                                         xw3[:, c, :],
                                         start=(c == 0), stop=(c == DC - 1))
                    for c in range(DC):
                        nc.tensor.matmul(kps[:],
                                         wslice(wk_sb, c, pr * P, (pr + 1) * P),
                                         xw3[:, c, :],
                                         start=(c == 0), stop=(c == DC - 1))
                    nc.vector.tensor_scalar_add(qt_tiles[pr][:, sl], qps[:],
                                                bq_sb[:, pr:pr + 1])
                    nc.vector.tensor_scalar_add(kt_tiles[pr][:, sl], kps[:],
                                                bk_sb[:, pr:pr + 1])
                for s4 in range(QB // P):
                    k = w * (QB // P) + s4
                    vps = ps_y.tile([P, HD], F32, tag="y")
                    for c in range(DC):
                        nc.tensor.matmul(vps[:], xw3[:, c, bass.ts(s4, P)],
                                         wslice(wv_sb, c, 0, HD),
                                         start=(c == 0), stop=(c == DC - 1))
                    v3 = v_tiles[k][:].rearrange("p (h v) -> p h v", h=HLOC)
                    nc.vector.tensor_add(v3[:, :, 0:DK],
                                         vps[:].rearrange("p (h d) -> p h d", h=HLOC),
                                         bv_sb[:].rearrange("p (h d) -> p h d", h=HLOC))
                    nc.vector.tensor_copy(v3[:, :, DK:VW], ones_sb[:].unsqueeze(2))

        # ---- phase 2: attention + out-projection per q-block ----
        with tc.tile_pool(name="wop", bufs=1) as wop, \
             tc.tile_pool(name="pexp", bufs=3) as pexp, \
             tc.tile_pool(name="otp", bufs=NPAIR + 1) as otp, \
             tc.tile_pool(name="misc", bufs=2) as misc:

            # Wo: [HD, D] -> [P, NPAIR, D] (pair-stacked rows)
            wo_sb = wop.tile([P, NPAIR * D], F32R, tag="wo")
            nc.sync.dma_start(
                wo_sb[:].rearrange("p (r n) -> p r n", r=NPAIR),
                Wo.ap().bitcast(F32R).rearrange("(r p) n -> p r n", p=P))

            yt_dram3 = YT.ap().rearrange("(n p) s -> p n s", p=P)

            for qb in range(NQB):
                qsl = bass.ts(qb, QB)
                ot_tiles = []
                for pr in range(NPAIR):
                    qt = qt_tiles[pr]
                    kt = kt_tiles[pr]
                    oa = ps_o.tile([VW, QB], F32, tag="oa")
                    ob = ps_o2.tile([VW, QB], F32, tag="ob")
                    for kc in range(KC):
                        ksl = bass.ts(kc, P)
                        sa = ps_a.tile([P, QB], F32, tag="sa")
                        sb = ps_b.tile([P, QB], F32, tag="sb")
                        nc.tensor.matmul(sa[:], kt[0:DK, ksl], qt[0:DK, qsl],
                                         start=True, stop=True)
                        nc.tensor.matmul(sb[:], kt[DK:P, ksl], qt[DK:P, qsl],
                                         start=True, stop=True)
                        ea = pexp.tile([P, QB], F32R, tag="ea")
                        eb = pexp.tile([P, QB], F32R, tag="eb")
                        nc.scalar.activation(ea[:], sa[:],
                                             mybir.ActivationFunctionType.Exp,
                                             scale=float(EXP_SCALE))
                        nc.scalar.activation(eb[:], sb[:],
                                             mybir.ActivationFunctionType.Exp,
                                             scale=float(EXP_SCALE))
                        vt = v_tiles[kc]
                        ha, hb = 2 * pr, 2 * pr + 1
                        nc.tensor.matmul(oa[:], vt[:, ha * VW:(ha + 1) * VW],
                                         ea[:], start=(kc == 0),
                                         stop=(kc == KC - 1))
                        nc.tensor.matmul(ob[:], vt[:, hb * VW:(hb + 1) * VW],
                                         eb[:], start=(kc == 0),
                                         stop=(kc == KC - 1))

                    # normalize rows 0:DK by row DK (the ones-column sums)
                    ra = misc.tile([1, QB], F32, tag="ra")
                    rb = misc.tile([1, QB], F32, tag="rb")
                    nc.vector.reciprocal(ra[:], oa[DK:VW, :])
                    nc.vector.reciprocal(rb[:], ob[DK:VW, :])
                    bc = misc.tile([P, QB], F32, tag="bc")
                    nc.gpsimd.partition_broadcast(bc[0:DK, :], ra[:], channels=DK)
                    nc.sync.dma_start(bc[DK:P, :],
                                      rb[:].unsqueeze(1).to_broadcast((1, DK, QB)))
                    ot = otp.tile([P, QB], F32R, tag="ot")
                    nc.vector.tensor_mul(ot[0:DK, :], oa[0:DK, :], bc[0:DK, :])
                    nc.vector.tensor_mul(ot[DK:P, :], ob[0:DK, :], bc[DK:P, :])
                    ot_tiles.append(ot)

                for n in range(NOC):
                    yps = ps_y.tile([P, QB], F32, tag="y")
                    for pr in range(NPAIR):
                        nc.tensor.matmul(
                            yps[:],
                            wo_sb[:, pr * D + n * P: pr * D + (n + 1) * P],
                            ot_tiles[pr][:],
                            start=(pr == 0), stop=(pr == NPAIR - 1))
                    ysb = misc.tile([P, QB], F32, tag="ysb")
                    nc.vector.tensor_scalar_add(ysb[:], yps[:], bo_sb[:, n:n + 1])
                    nc.sync.dma_start(yt_dram3[:, n, qsl], ysb[:])

    nc.compile()
    return nc


_CACHE = {}


def _get_nc():
    if "nc" not in _CACHE:
        _CACHE["nc"] = build_bass()
    return _CACHE["nc"]


def host_prep(x, Wq, bq, Wk, bk, Wv, bv, Wo, bo):
    """Build the 8 per-core input maps."""
    NPAIR = HL // 2
    NOC = D_MODEL // P
    in_maps = []
    for core in range(N_CORES):
        b, g = divmod(core, 2)
        lo, hi = g * HL * DK, (g + 1) * HL * DK
        in_maps.append({
            "xT": np.ascontiguousarray(x[b].T),
            "Wq": np.ascontiguousarray(Wq[:, lo:hi]),
            "Wk": np.ascontiguousarray(Wk[:, lo:hi]),
            "Wv": np.ascontiguousarray(Wv[:, lo:hi]),
            "Wo": np.ascontiguousarray(Wo[lo:hi, :]),
            "bq_t": np.ascontiguousarray(bq[lo:hi].reshape(NPAIR, P).T),
            "bk_t": np.ascontiguousarray(bk[lo:hi].reshape(NPAIR, P).T),
            "bv_bc": np.broadcast_to(bv[lo:hi], (P, HL * DK)).copy(),
            "bo_t": np.ascontiguousarray((bo * 0.5).reshape(NOC, P).T),
        })
    return in_maps


def host_gather(results):
    """Sum head-group partials and transpose back to [B, S, D]."""
    out = np.empty((BATCH, SEQ, D_MODEL), dtype=np.float32)
    for b in range(BATCH):
        yt = results[2 * b]["YT"] + results[2 * b + 1]["YT"]
        out[b] = yt.T
    return out


def kernel(x, Wq, bq, Wk, bk, Wv, bv, Wo, bo):
    nc = _get_nc()
    in_maps = host_prep(x, Wq, bq, Wk, bk, Wv, bv, Wo, bo)
    res = run_bass_kernel_spmd(nc, in_maps, core_ids=list(range(N_CORES)))
    return host_gather(res.results)


# revision 10
# speedup vs baseline: 144.6075x; 144.6075x over previous
"""Multi-head self-attention kernel for 8 Trainium2 NeuronCores.

Sharding: core c = (b, g) with b = batch index (4), g = head-group (2).
Each core computes attention for one batch element and 8 of the 16 heads,
including its slice of the QKV projections and a partial out-projection
(Y_partial = O_heads @ Wo[rows of its heads]).  The host sums the two
head-group partials per batch and transposes (the device produces Y^T).

On-device layout is fully "transposed": x^T [D, S] in, Q^T/K^T [dk, S],
scores S^T = K_h Q_h^T [k, q] (softmax along partitions via a ones-column
appended to V: the PV matmul O^T_aug = [V|1]^T P^T yields the softmax
denominator in its last row), output Y^T [D, S].
"""

import sys

sys.path.insert(0, "/opt/trn_rl_repo")

from contextlib import ExitStack

import numpy as np

import concourse.bass as bass
import concourse.tile as tile
from concourse import bacc, mybir
from concourse.bass_utils import run_bass_kernel_spmd

F32 = mybir.dt.float32
F32R = mybir.dt.float32r
P = 128  # SBUF partitions

D_MODEL = 1024
NHEAD = 16
DK = D_MODEL // NHEAD  # 64
BATCH = 4
SEQ = 2048
N_CORES = 8
HL = NHEAD // 2  # heads per core (head-group of 8)


def build_bass(D=D_MODEL, S=SEQ, HLOC=HL, QB=512, repeat=1):
    """Build the per-core Bass program (same program on all 8 cores)."""
    DC = D // P           # d_model chunks (contraction for projections)
    KC = S // P           # key chunks
    NQB = S // QB         # q blocks
    NPAIR = HLOC // 2     # head pairs
    HD = HLOC * DK        # local head dim total (512)
    VW = DK + 1           # V columns per head incl. ones column
    NOC = D // P          # out-dim chunks
    EXP_SCALE = 1.0 / np.sqrt(DK)

    nc = bacc.Bacc("TRN2", target_bir_lowering=False, debug=False,
                   num_devices=N_CORES)

    xT = nc.dram_tensor("xT", [D, S], F32, kind="ExternalInput")
    Wq = nc.dram_tensor("Wq", [D, HD], F32, kind="ExternalInput")
    Wk = nc.dram_tensor("Wk", [D, HD], F32, kind="ExternalInput")
    Wv = nc.dram_tensor("Wv", [D, HD], F32, kind="ExternalInput")
    Wo = nc.dram_tensor("Wo", [HD, D], F32, kind="ExternalInput")
    bq_t = nc.dram_tensor("bq_t", [P, NPAIR], F32, kind="ExternalInput")
    bk_t = nc.dram_tensor("bk_t", [P, NPAIR], F32, kind="ExternalInput")
    bv_bc = nc.dram_tensor("bv_bc", [P, HD], F32, kind="ExternalInput")
    bo_t = nc.dram_tensor("bo_t", [P, NOC], F32, kind="ExternalInput")
    YT = nc.dram_tensor("YT", [D, S], F32, kind="ExternalOutput")

    with tile.TileContext(nc) as tc, ExitStack() as ctx:
        consts = ctx.enter_context(tc.tile_pool(name="consts", bufs=1))
        ktv = ctx.enter_context(tc.tile_pool(name="ktv", bufs=1))
        ps_a = ctx.enter_context(tc.tile_pool(name="ps_a", bufs=2, space="PSUM"))
        ps_b = ctx.enter_context(tc.tile_pool(name="ps_b", bufs=2, space="PSUM"))
        # oa/ob/y accumulators share one 4-slot pool: the normalizing pair's
        # two PSUM banks coexist with the accumulating pair's two.
        ps_acc = ctx.enter_context(tc.tile_pool(name="ps_acc", bufs=4,
                                                space="PSUM"))

        # ---- constants ----
        bq_sb = consts.tile([P, NPAIR], F32, tag="bq")
        bk_sb = consts.tile([P, NPAIR], F32, tag="bk")
        bv_sb = consts.tile([P, HD], F32, tag="bv")
        bo_sb = consts.tile([P, NOC], F32, tag="bo")
        nc.sync.dma_start(bq_sb[:], bq_t.ap())
        nc.sync.dma_start(bk_sb[:], bk_t.ap())
        nc.sync.dma_start(bv_sb[:], bv_bc.ap())
        nc.sync.dma_start(bo_sb[:], bo_t.ap())

        # warm the ACT exp table early
        warm = consts.tile([1, 2], F32, tag="warm")
        nc.gpsimd.memset(warm[0:1, 0:1], 0.0)
        nc.scalar.activation(warm[0:1, 1:2], warm[0:1, 0:1],
                             mybir.ActivationFunctionType.Exp)

        ones_sb = consts.tile([P, HLOC], F32, tag="ones")
        nc.vector.memset(ones_sb[:], 1.0)

        xt_dram3 = xT.ap().bitcast(F32R).rearrange("(c p) s -> p c s", p=P)
        yt_dram3 = YT.ap().rearrange("(n p) s -> p n s", p=P)

        for _rep in range(repeat):
            emit_body(nc, tc, ktv, ps_a, ps_b, ps_acc,
                      dict(D=D, S=S, HLOC=HLOC, QB=QB, DC=DC, KC=KC, NQB=NQB,
                           NPAIR=NPAIR, HD=HD, VW=VW, NOC=NOC,
                           EXP_SCALE=EXP_SCALE),
                      xt_dram3, yt_dram3, Wq, Wk, Wv, Wo,
                      bq_sb, bk_sb, bv_sb, bo_sb, ones_sb)

    nc.compile()
    return nc


def emit_body(nc, tc, ktv, ps_a, ps_b, ps_acc, cfg, xt_dram3, yt_dram3,
              Wq, Wk, Wv, Wo, bq_sb, bk_sb, bv_sb, bo_sb, ones_sb):
    D, S, HLOC, QB = cfg["D"], cfg["S"], cfg["HLOC"], cfg["QB"]
    DC, KC, NQB, NPAIR = cfg["DC"], cfg["KC"], cfg["NQB"], cfg["NPAIR"]
    HD, VW, NOC, EXP_SCALE = cfg["HD"], cfg["VW"], cfg["NOC"], cfg["EXP_SCALE"]

    # resident outputs of phase 1
    qt_tiles = [ktv.tile([P, S], F32R, tag=f"qt{p_}", name=f"qt{p_}")
                for p_ in range(NPAIR)]
    kt_tiles = [ktv.tile([P, S], F32R, tag=f"kt{p_}", name=f"kt{p_}")
                for p_ in range(NPAIR)]
    v_tiles = [ktv.tile([P, HLOC * VW], F32R, tag=f"v{k}", name=f"v{k}")
               for k in range(KC)]

    # ---- phase 1: Q^T, K^T, V projections (x^T + W streamed) ----
    with tc.tile_pool(name="wqkv", bufs=1) as wpool, \
         tc.tile_pool(name="xwin", bufs=2) as xpool:

        def load_w(name, dram, cols):
            t = wpool.tile([P, DC * cols], F32R, tag=name, name=name)
            nc.sync.dma_start(
                t[:].rearrange("p (c n) -> p c n", c=DC),
                dram.ap().bitcast(F32R).rearrange("(c p) n -> p c n", p=P))
            return t

        wq_sb = load_w("wq", Wq, HD)
        wk_sb = load_w("wk", Wk, HD)
        wv_sb = load_w("wv", Wv, HD)

        def wslice(wt, c, lo, hi):
            return wt[:, c * HD + lo: c * HD + hi]

        for w in range(NQB):
            sl = bass.ts(w, QB)
            xw = xpool.tile([P, DC * QB], F32R, tag="xw", name="xw")
            xw3 = xw[:].rearrange("p (c s) -> p c s", c=DC)
            nc.sync.dma_start(xw3[:], xt_dram3[:, :, sl])

            for pr in range(NPAIR):
                qps = ps_a.tile([P, QB], F32, tag="sa", name="qps")
                kps = ps_b.tile([P, QB], F32, tag="sb", name="kps")
                for c in range(DC):
                    nc.tensor.matmul(qps[:],
                                     wslice(wq_sb, c, pr * P, (pr + 1) * P),
                                     xw3[:, c, :],
                                     start=(c == 0), stop=(c == DC - 1))
                for c in range(DC):
                    nc.tensor.matmul(kps[:],
                                     wslice(wk_sb, c, pr * P, (pr + 1) * P),
                                     xw3[:, c, :],
                                     start=(c == 0), stop=(c == DC - 1))
                nc.vector.tensor_scalar_add(qt_tiles[pr][:, sl], qps[:],
                                            bq_sb[:, pr:pr + 1])
                nc.vector.tensor_scalar_add(kt_tiles[pr][:, sl], kps[:],
                                            bk_sb[:, pr:pr + 1])
            for s4 in range(QB // P):
                k = w * (QB // P) + s4
                vps = ps_acc.tile([P, HD], F32, tag="acc", name="vps")
                for c in range(DC):
                    nc.tensor.matmul(vps[:], xw3[:, c, bass.ts(s4, P)],
                                     wslice(wv_sb, c, 0, HD),
                                     start=(c == 0), stop=(c == DC - 1))
                v3 = v_tiles[k][:].rearrange("p (h v) -> p h v", h=HLOC)
                nc.vector.tensor_add(v3[:, :, 0:DK],
                                     vps[:].rearrange("p (h d) -> p h d", h=HLOC),
                                     bv_sb[:].rearrange("p (h d) -> p h d", h=HLOC))
                nc.vector.tensor_copy(v3[:, :, DK:VW], ones_sb[:].unsqueeze(2))

    # ---- phase 2: attention + out-projection per q-block ----
    with tc.tile_pool(name="wop", bufs=1) as wop, \
         tc.tile_pool(name="pexp", bufs=3) as pexp, \
         tc.tile_pool(name="otp", bufs=NPAIR + 1) as otp, \
         tc.tile_pool(name="misc", bufs=2) as misc:

        # Wo: [HD, D] -> [P, NPAIR, D] (pair-stacked rows)
        wo_sb = wop.tile([P, NPAIR * D], F32R, tag="wo", name="wo")
        nc.sync.dma_start(
            wo_sb[:].rearrange("p (r n) -> p r n", r=NPAIR),
            Wo.ap().bitcast(F32R).rearrange("(r p) n -> p r n", p=P))

        for qb in range(NQB):
            qsl = bass.ts(qb, QB)
            ot_tiles = []
            for pr in range(NPAIR):
                qt = qt_tiles[pr]
                kt = kt_tiles[pr]
                oa = ps_acc.tile([VW, QB], F32, tag="acc", name="oa")
                ob = ps_acc.tile([VW, QB], F32, tag="acc", name="ob")
                for kc in range(KC):
                    ksl = bass.ts(kc, P)
                    sa = ps_a.tile([P, QB], F32, tag="sa", name="sa")
                    sb = ps_b.tile([P, QB], F32, tag="sb", name="sb")
                    nc.tensor.matmul(sa[:], kt[0:DK, ksl], qt[0:DK, qsl],
                                     start=True, stop=True)
                    nc.tensor.matmul(sb[:], kt[DK:P, ksl], qt[DK:P, qsl],
                                     start=True, stop=True)
                    ea = pexp.tile([P, QB], F32R, tag="ea", name="ea")
                    eb = pexp.tile([P, QB], F32R, tag="eb", name="eb")
                    nc.scalar.activation(ea[:], sa[:],
                                         mybir.ActivationFunctionType.Exp,
                                         scale=float(EXP_SCALE))
                    nc.scalar.activation(eb[:], sb[:],
                                         mybir.ActivationFunctionType.Exp,
                                         scale=float(EXP_SCALE))
                    vt = v_tiles[kc]
                    ha, hb = 2 * pr, 2 * pr + 1
                    nc.tensor.matmul(oa[:], vt[:, ha * VW:(ha + 1) * VW],
                                     ea[:], start=(kc == 0),
                                     stop=(kc == KC - 1))
                    nc.tensor.matmul(ob[:], vt[:, hb * VW:(hb + 1) * VW],
                                     eb[:], start=(kc == 0),
                                     stop=(kc == KC - 1))

                # normalize rows 0:DK by row DK (the ones-column sums)
                ra = misc.tile([1, QB], F32, tag="ra", name="ra")
                rb = misc.tile([1, QB], F32, tag="rb", name="rb")
                nc.vector.reciprocal(ra[:], oa[DK:VW, :])
                nc.vector.reciprocal(rb[:], ob[DK:VW, :])
                bc = misc.tile([P, QB], F32, tag="bc", name="bc")
                nc.gpsimd.partition_broadcast(bc[0:DK, :], ra[:], channels=DK)
                nc.sync.dma_start(bc[DK:P, :],
                                  rb[:].unsqueeze(1).to_broadcast((1, DK, QB)))
                ot = otp.tile([P, QB], F32R, tag="ot", name="ot")
                nc.vector.tensor_mul(ot[0:DK, :], oa[0:DK, :], bc[0:DK, :])
                nc.vector.tensor_mul(ot[DK:P, :], ob[0:DK, :], bc[DK:P, :])
                ot_tiles.append(ot)

            for n in range(NOC):
                yps = ps_acc.tile([P, QB], F32, tag="acc", name="yps")
                for pr in range(NPAIR):
                    nc.tensor.matmul(
                        yps[:],
                        wo_sb[:, pr * D + n * P: pr * D + (n + 1) * P],
                        ot_tiles[pr][:],
                        start=(pr == 0), stop=(pr == NPAIR - 1))
                ysb = misc.tile([P, QB], F32, tag="ysb", name="ysb")
                nc.vector.tensor_scalar_add(ysb[:], yps[:], bo_sb[:, n:n + 1])
                nc.sync.dma_start(yt_dram3[:, n, qsl], ysb[:])


_CACHE = {}


def _get_nc():
    if "nc" not in _CACHE:
        _CACHE["nc"] = build_bass()
    return _CACHE["nc"]


def host_prep(x, Wq, bq, Wk, bk, Wv, bv, Wo, bo):
    """Build the 8 per-core input maps."""
    NPAIR = HL // 2
    NOC = D_MODEL // P
    in_maps = []
    for core in range(N_CORES):
        b, g = divmod(core, 2)
        lo, hi = g * HL * DK, (g + 1) * HL * DK
        in_maps.append({
            "xT": np.ascontiguousarray(x[b].T),
            "Wq": np.ascontiguousarray(Wq[:, lo:hi]),
            "Wk": np.ascontiguousarray(Wk[:, lo:hi]),
            "Wv": np.ascontiguousarray(Wv[:, lo:hi]),
            "Wo": np.ascontiguousarray(Wo[lo:hi, :]),
            "bq_t": np.ascontiguousarray(bq[lo:hi].reshape(NPAIR, P).T),
            "bk_t": np.ascontiguousarray(bk[lo:hi].reshape(NPAIR, P).T),
            "bv_bc": np.broadcast_to(bv[lo:hi], (P, HL * DK)).copy(),
            "bo_t": np.ascontiguousarray((bo * 0.5).reshape(NOC, P).T),
        })
    return in_maps


def host_gather(results):
    """Sum head-group partials and transpose back to [B, S, D]."""
    out = np.empty((BATCH, SEQ, D_MODEL), dtype=np.float32)
    for b in range(BATCH):
        yt = results[2 * b]["YT"] + results[2 * b + 1]["YT"]
        out[b] = yt.T
    return out


def kernel(x, Wq, bq, Wk, bk, Wv, bv, Wo, bo):
    nc = _get_nc()
    in_maps = host_prep(x, Wq, bq, Wk, bk, Wv, bv, Wo, bo)
    res = run_bass_kernel_spmd(nc, in_maps, core_ids=list(range(N_CORES)))
    return host_gather(res.results)
